# revision 27
# baseline (speedup 1.0000x reference)
"""MHSA block (patch-embed conv + relative-pos attention + MLP) on 8 NeuronCores.

Sharding: pure data-parallel over batch (64 images -> 8 per core), weights
replicated. Host does only layout prep (transposes/casts/rel-pos gather);
all model compute runs on-device via Bass/Tile.
"""
import numpy as np
import ml_dtypes
import concourse.bass as bass
import concourse.bacc as bacc
import concourse.tile as tile
from concourse import mybir
from concourse import bass_utils
from concourse.masks import make_identity

BF = ml_dtypes.bfloat16
B, CIN, D, HEADS, HD = 64, 384, 768, 12, 64
GS, ET, N = 16, 1, 257
BL = B // 8              # images per core
NT = BL * N              # 2056 packed tokens per core
MLP = 4 * D
CHUNKS = [(i * 128, min(128, NT - i * 128)) for i in range(17)]
COLT = [(c, min(512, NT - c)) for c in range(0, NT, 512)]
MCH = [(0, 128), (128, 128), (256, 1)]

_CACHE = {}


def _rel_bias(rpb_table):
    coords = np.stack(np.meshgrid(np.arange(GS), np.arange(GS), indexing='ij'))
    cf = coords.reshape(2, -1)
    rel = (cf[:, :, None] - cf[:, None, :]).transpose(1, 2, 0)
    rel[:, :, 0] += GS - 1
    rel[:, :, 1] += GS - 1
    rel[:, :, 0] *= 2 * GS - 1
    idx = rel.sum(-1)
    out = np.zeros((N, N), dtype=np.int32)
    out[ET:, ET:] = idx
    bias = rpb_table[out]                    # [N, N, HEADS]
    return bias.transpose(2, 0, 1).astype(np.float32)   # [HEADS, N, N]


def _ln_pair(tc, nc, pools, xt, ts):
    """mean/rstd of xt[:ts, :768] -> (mean, rstd) [ts,1] f32 tiles."""
    st = pools.tile([128, 3, nc.vector.BN_STATS_DIM], mybir.dt.float32, tag="lnst")
    xg = xt.rearrange("p (n f) -> p n f", f=256)
    for i in range(3):
        nc.vector.bn_stats(out=st[:ts, i], in_=xg[:ts, i])
    mv = pools.tile([128, nc.vector.BN_AGGR_DIM], mybir.dt.float32, tag="lnmv")
    nc.vector.bn_aggr(out=mv[:ts], in_=st[:ts])
    eps = pools.tile([128, 1], mybir.dt.float32, tag="lneps")
    nc.vector.memset(eps, 1e-5)
    rs = pools.tile([128, 1], mybir.dt.float32, tag="lnrs")
    nc.scalar.activation(out=rs[:ts], in_=mv[:ts, 1:2],
                         func=mybir.ActivationFunctionType.Sqrt, bias=eps[:ts])
    nc.vector.reciprocal(out=rs[:ts], in_=rs[:ts])
    return mv, rs


def build():
    nc = bacc.Bacc("TRN2", target_bir_lowering=False, debug=False)
    f32, bf16 = mybir.dt.float32, mybir.dt.bfloat16
    di = lambda n, s, d: nc.dram_tensor(n, s, d, kind="ExternalInput").ap()
    x_in = di("x_in", [BL, 3, 128, 32, 32], bf16)
    convw = di("convw", [27, 128, 768], bf16)
    convb_bc = di("convb_bc", [128, 768], f32)
    peg_bc = di("peg_bc", [128, 768], f32)
    geo2 = di("geo2", [2, 128, 768], f32)
    y0row = di("y0row", [1, 768], f32)
    qkvw = di("qkvw", [6, 128, 2304], bf16)
    qkvb_t = di("qkvb_t", [128, 18], f32)
    projw = di("projw", [6, 128, 768], bf16)
    projb_bc = di("projb_bc", [128, 768], f32)
    fc1w = di("fc1w", [6, 128, MLP], bf16)
    fc1b_t = di("fc1b_t", [128, 24], f32)
    fc2w = di("fc2w", [24, 128, 768], bf16)
    fc2b_bc = di("fc2b_bc", [128, 768], f32)
    biasT = di("biasT", [12, 128, 3, 257], bf16)
    ones_d = di("ones_d", [1, 257], bf16)
    e2_d = di("e2_d", [2, 128], bf16)
    out_d = nc.dram_tensor("out_d", [NT, 768], f32, kind="ExternalOutput").ap()

    with tile.TileContext(nc) as tc:
        with tc.tile_pool(name="dram", bufs=1, space="DRAM") as dpool:
            y_d = dpool.tile([NT, 768], f32)
            hT_d = dpool.tile([6, 128, NT], bf16)
            qkT_d = dpool.tile([18, 128, NT], bf16)
            oT_d = dpool.tile([6, 128, NT], bf16)
            y2_d = dpool.tile([NT, 768], f32)
            den_d = dpool.tile([12, NT], bf16)
            h2T_d = dpool.tile([6, 128, NT], bf16)
            h3T_d = dpool.tile([24, 128, NT], bf16)

            # ---------------- Phase 1: conv + peLN + geo -> y_d ----------------
            with tc.tile_pool(name="cw", bufs=1) as cw, \
                 tc.tile_pool(name="cx", bufs=2) as cx, \
                 tc.tile_pool(name="cps", bufs=4, space="PSUM") as cps, \
                 tc.tile_pool(name="cy", bufs=3) as cy:
                wsb = cw.tile([128, 27, 768], bf16)
                for i in range(27):
                    nc.sync.dma_start(out=wsb[:, i], in_=convw[i])
                cbc = cw.tile([128, 768], f32)
                nc.sync.dma_start(out=cbc, in_=convb_bc)
                pgc = cw.tile([128, 768], f32)
                nc.sync.dma_start(out=pgc, in_=peg_bc)
                gsb = cw.tile([128, 2, 768], f32)
                for t in range(2):
                    nc.sync.dma_start(out=gsb[:, t], in_=geo2[t])
                y0sb = cw.tile([1, 768], f32)
                nc.sync.dma_start(out=y0sb, in_=y0row)
                for b in range(BL):
                    nc.sync.dma_start(out=y_d[b * N:b * N + 1, :], in_=y0sb)
                for b in range(BL):
                    xp = cx.tile([128, 3, 1089], bf16, tag="xpad")
                    nc.vector.memset(xp, 0.0)
                    for c in range(3):
                        dst = bass.AP(tensor=xp.tensor, offset=xp.offset + c * 1089 + 34,
                                      ap=[xp.ap[0], [33, 32], [1, 32]])
                        nc.sync.dma_start(out=dst, in_=x_in[b, c])
                    for t in range(2):
                        col = cx.tile([128, 27, 128], bf16, tag="col")
                        for kh in range(3):
                            for kw in range(3):
                                for c in range(3):
                                    idx = (kh * 3 + kw) * 3 + c
                                    src = bass.AP(
                                        tensor=xp.tensor,
                                        offset=xp.offset + c * 1089 + (16 * t + kh) * 33 + kw,
                                        ap=[xp.ap[0], [66, 8], [2, 16]])
                                    if c == 0:
                                        nc.vector.tensor_copy(col[:, idx], src)
                                    elif c == 1:
                                        nc.gpsimd.tensor_copy(col[:, idx], src)
                                    else:
                                        nc.scalar.copy(col[:, idx], src)
                        yt = cy.tile([128, 768], f32, tag="yt")
                        for nh in range(2):
                            ps = cps.tile([128, 384], f32, tag="cpsum")
                            for i in range(27):
                                nc.tensor.matmul(ps, col[:, i], wsb[:, i, nh * 384:(nh + 1) * 384],
                                                 start=(i == 0), stop=(i == 26))
                            nc.vector.tensor_add(yt[:, nh * 384:(nh + 1) * 384], ps, cbc[:, nh * 384:(nh + 1) * 384])
                        mv, rs = _ln_pair(tc, nc, cy, yt, 128)
                        nc.vector.tensor_scalar(out=yt, in0=yt, scalar1=mv[:, 0:1], scalar2=rs,
                                                op0=mybir.AluOpType.subtract, op1=mybir.AluOpType.mult)
                        nc.gpsimd.tensor_mul(yt, yt, pgc)
                        nc.vector.tensor_add(yt, yt, gsb[:, t])
                        r0 = b * N + 1 + t * 128
                        nc.sync.dma_start(out=y_d[r0:r0 + 128, :], in_=yt)

            # ---------------- Phase 2: LN1 + transpose -> hT_d ----------------
            with tc.tile_pool(name="l1", bufs=3) as l1, \
                 tc.tile_pool(name="l1c", bufs=1) as l1c, \
                 tc.tile_pool(name="l1p", bufs=4, space="PSUM") as l1p:
                idb = l1c.tile([128, 128], bf16)
                make_identity(nc, idb)
                for (t0, ts) in CHUNKS:
                    yt = l1.tile([128, 768], f32, tag="l1y")
                    nc.sync.dma_start(out=yt[:ts], in_=y_d[t0:t0 + ts, :])
                    mv, rs = _ln_pair(tc, nc, l1, yt, ts)
                    hb = l1.tile([128, 768], bf16, tag="l1h")
                    nc.vector.tensor_scalar(out=hb[:ts], in0=yt[:ts], scalar1=mv[:ts, 0:1], scalar2=rs[:ts],
                                            op0=mybir.AluOpType.subtract, op1=mybir.AluOpType.mult)
                    for k in range(6):
                        tp = l1p.tile([128, 128], bf16, tag="l1t")
                        nc.tensor.transpose(tp[:, :ts], hb[:ts, k * 128:(k + 1) * 128], idb[:ts, :ts])
                        st = l1.tile([128, 128], bf16, tag="l1s")
                        nc.vector.tensor_copy(st[:, :ts], tp[:, :ts])
                        nc.sync.dma_start(out=hT_d[k, :, t0:t0 + ts], in_=st[:, :ts])

            # ---------------- Phase 3: QKV -> qkT_d ----------------
            with tc.tile_pool(name="qw", bufs=1) as qw, \
                 tc.tile_pool(name="qa", bufs=3) as qa, \
                 tc.tile_pool(name="qp", bufs=8, space="PSUM") as qp:
                wq = qw.tile([128, 6, 2304], bf16)
                for k in range(6):
                    nc.sync.dma_start(out=wq[:, k], in_=qkvw[k])
                qb = qw.tile([128, 18], f32)
                nc.sync.dma_start(out=qb, in_=qkvb_t)
                for (c0, cs) in COLT:
                    ht = qa.tile([128, 6, 512], bf16, tag="qh")
                    for k in range(6):
                        nc.sync.dma_start(out=ht[:, k, :cs], in_=hT_d[k, :, c0:c0 + cs])
                    for dch in range(18):
                        ps = qp.tile([128, 512], f32, tag="qps")
                        for k in range(6):
                            nc.tensor.matmul(ps[:, :cs], wq[:, k, dch * 128:(dch + 1) * 128],
                                             ht[:, k, :cs], start=(k == 0), stop=(k == 5))
                        ev = qa.tile([128, 512], bf16, tag="qev")
                        nc.vector.tensor_scalar_add(out=ev[:, :cs], in0=ps[:, :cs], scalar1=qb[:, dch:dch + 1])
                        nc.sync.dma_start(out=qkT_d[dch, :, c0:c0 + cs], in_=ev[:, :cs])

            # ---------------- Phase 4: attention (S^T orientation) -> oT_d ----------------
            # S^T [keys, queries] = K Q^T + bias^T (bias injected via identity-
            # stationary matmul into the same PSUM). The softmax denominator
            # comes free from a ones row appended to V before its transpose.
            # o^T is written UNNORMALIZED; denominators go to den_d and the
            # per-query 1/denom scaling happens batched in phase 5.
            with tc.tile_pool(name="ac", bufs=1) as ac, \
                 tc.tile_pool(name="ab", bufs=2) as ab, \
                 tc.tile_pool(name="aw", bufs=3) as aw, \
                 tc.tile_pool(name="aps", bufs=3, space="PSUM") as aps, \
                 tc.tile_pool(name="apt", bufs=2, space="PSUM") as apt, \
                 tc.tile_pool(name="apo", bufs=1, space="PSUM") as apo:
                idb = ac.tile([128, 128], bf16)
                make_identity(nc, idb)
                for h in range(12):
                    bsb = ab.tile([128, 3, 257], bf16, tag="bias")
                    nc.sync.dma_start(out=bsb, in_=biasT[h])
                    for b in range(BL):
                        po = (h % 2) * 64
                        qt = aw.tile([64, 257], bf16, tag="qt")
                        kt = aw.tile([64, 257], bf16, tag="kt")
                        vt = aw.tile([65, 257], bf16, tag="vt")
                        nc.sync.dma_start(out=qt, in_=qkT_d[h // 2, po:po + 64, b * N:b * N + N])
                        nc.sync.dma_start(out=kt, in_=qkT_d[6 + h // 2, po:po + 64, b * N:b * N + N])
                        nc.sync.dma_start(out=vt[:64], in_=qkT_d[12 + h // 2, po:po + 64, b * N:b * N + N])
                        nc.sync.dma_start(out=vt[64:65], in_=ones_d)
                        vsb = aw.tile([128, 3, 65], bf16, tag="vsb")
                        for mi, (mo, ms) in enumerate(MCH):
                            tp = apt.tile([128, 65], bf16, tag="vtp")
                            nc.tensor.transpose(tp[:ms], vt[:, mo:mo + ms], idb[:65, :65])
                            nc.vector.tensor_copy(vsb[:ms, mi], tp[:ms])
                        pr = aw.tile([128, 3, 257], bf16, tag="pr")
                        for mi, (mo, ms) in enumerate(MCH):
                            sp = aps.tile([128, 257], f32, tag="sps")
                            nc.tensor.matmul(sp[:ms], kt[:, mo:mo + ms], qt,
                                             start=True, stop=True)
                            nc.scalar.activation(pr[:ms, mi], sp[:ms],
                                                 mybir.ActivationFunctionType.Exp)
                            # exp(s + b) = exp(s) * exp(b); bsb holds exp(bias^T)
                            nc.gpsimd.tensor_mul(pr[:ms, mi], pr[:ms, mi], bsb[:ms, mi])
                        op = apo.tile([65, 257], f32, tag="ops", bufs=2)
                        for mi, (mo, ms) in enumerate(MCH):
                            nc.tensor.matmul(op, vsb[:ms, mi], pr[:ms, mi],
                                             start=(mi == 0), stop=(mi == 2))
                        oe = aw.tile([65, 257], bf16, tag="oe")
                        nc.vector.tensor_copy(oe, op)
                        nc.sync.dma_start(out=oT_d[h // 2, po:po + 64, b * N:b * N + N], in_=oe[:64])
                        nc.sync.dma_start(out=den_d[h:h + 1, b * N:b * N + N], in_=oe[64:65])

            # ---------------- Phase 5: proj + residual + LN2 + T -> y2_d, h2T_d ----------------
            with tc.tile_pool(name="pw", bufs=1) as pw, \
                 tc.tile_pool(name="pa", bufs=3) as pa, \
                 tc.tile_pool(name="pp", bufs=3, space="PSUM") as pp:
                wp = pw.tile([128, 6, 768], bf16)
                for k in range(6):
                    nc.sync.dma_start(out=wp[:, k], in_=projw[k])
                pbc = pw.tile([128, 768], f32)
                nc.sync.dma_start(out=pbc, in_=projb_bc)
                idb2 = pw.tile([128, 128], bf16)
                make_identity(nc, idb2)
                # batched softmax denominators: one reciprocal over [12, NT]
                dsb = pw.tile([12, NT], bf16)
                nc.sync.dma_start(out=dsb, in_=den_d)
                rcb = pw.tile([12, NT], f32)
                nc.vector.reciprocal(rcb, dsb)
                rch = pw.tile([12, NT], bf16)
                nc.vector.tensor_copy(rch, rcb)
                # head-pair rows (2k, 2k+1) at partitions (0, 1) for k=0..5
                den2 = pw.tile([2, 6, NT], bf16)
                for k in range(6):
                    nc.sync.dma_start(out=den2[0:1, k], in_=rch[2 * k:2 * k + 1])
                    nc.sync.dma_start(out=den2[1:2, k], in_=rch[2 * k + 1:2 * k + 2])
                e2 = pw.tile([2, 128], bf16)
                nc.sync.dma_start(out=e2, in_=e2_d)
                for (t0, ts) in CHUNKS:
                    ot = pa.tile([128, 6, 128], bf16, tag="pot")
                    for k in range(6):
                        nc.sync.dma_start(out=ot[:, k, :ts], in_=oT_d[k, :, t0:t0 + ts])
                    for k in range(6):
                        sden = pp.tile([128, 128], f32, tag="sden", bufs=2)
                        nc.tensor.matmul(sden[:, :ts], e2, den2[:, k, t0:t0 + ts],
                                         start=True, stop=True)
                        nc.vector.tensor_mul(ot[:, k, :ts], ot[:, k, :ts], sden[:, :ts])
                    yt = pa.tile([128, 768], f32, tag="py")
                    nc.sync.dma_start(out=yt[:ts], in_=y_d[t0:t0 + ts, :])
                    y2 = pa.tile([128, 768], f32, tag="py2")
                    for nh in range(2):
                        ps = pp.tile([128, 384], f32, tag="pps")
                        for k in range(6):
                            nc.tensor.matmul(ps[:ts], ot[:, k, :ts], wp[:, k, nh * 384:(nh + 1) * 384],
                                             start=(k == 0), stop=(k == 5))
                        nc.vector.tensor_add(y2[:ts, nh * 384:(nh + 1) * 384], ps[:ts],
                                             yt[:ts, nh * 384:(nh + 1) * 384])
                    nc.vector.tensor_add(y2[:ts], y2[:ts], pbc[:ts])
                    nc.sync.dma_start(out=y2_d[t0:t0 + ts, :], in_=y2[:ts])
                    mv, rs = _ln_pair(tc, nc, pa, y2, ts)
                    hb = pa.tile([128, 768], bf16, tag="ph2")
                    nc.vector.tensor_scalar(out=hb[:ts], in0=y2[:ts], scalar1=mv[:ts, 0:1], scalar2=rs[:ts],
                                            op0=mybir.AluOpType.subtract, op1=mybir.AluOpType.mult)
                    for k in range(6):
                        tp = pp.tile([128, 128], bf16, tag="ptr")
                        nc.tensor.transpose(tp[:, :ts], hb[:ts, k * 128:(k + 1) * 128], idb2[:ts, :ts])
                        st = pa.tile([128, 128], bf16, tag="pst")
                        nc.vector.tensor_copy(st[:, :ts], tp[:, :ts])
                        nc.sync.dma_start(out=h2T_d[k, :, t0:t0 + ts], in_=st[:, :ts])

            # ---------------- Phase 6: FC1 + gelu -> h3T_d ----------------
            with tc.tile_pool(name="f1w", bufs=1) as f1w, \
                 tc.tile_pool(name="f1a", bufs=3) as f1a, \
                 tc.tile_pool(name="f1p", bufs=8, space="PSUM") as f1p:
                w1 = f1w.tile([128, 6, MLP], bf16)
                for k in range(6):
                    nc.sync.dma_start(out=w1[:, k], in_=fc1w[k])
                b1 = f1w.tile([128, 24], f32)
                nc.sync.dma_start(out=b1, in_=fc1b_t)
                for (c0, cs) in COLT:
                    ht = f1a.tile([128, 6, 512], bf16, tag="f1h")
                    for k in range(6):
                        nc.sync.dma_start(out=ht[:, k, :cs], in_=h2T_d[k, :, c0:c0 + cs])
                    for dch in range(24):
                        ps = f1p.tile([128, 512], f32, tag="f1ps")
                        for k in range(6):
                            nc.tensor.matmul(ps[:, :cs], w1[:, k, dch * 128:(dch + 1) * 128],
                                             ht[:, k, :cs], start=(k == 0), stop=(k == 5))
                        ev = f1a.tile([128, 512], bf16, tag="f1ev")
                        nc.scalar.activation(ev[:, :cs], ps[:, :cs], mybir.ActivationFunctionType.Gelu,
                                             bias=b1[:, dch:dch + 1])
                        nc.sync.dma_start(out=h3T_d[dch, :, c0:c0 + cs], in_=ev[:, :cs])

            # ---------------- Phase 7: FC2 + residual -> out ----------------
            with tc.tile_pool(name="f2w", bufs=1) as f2w, \
                 tc.tile_pool(name="f2a", bufs=3) as f2a, \
                 tc.tile_pool(name="f2p", bufs=8, space="PSUM") as f2p:
                w2 = f2w.tile([128, 24, 768], bf16)
                for k in range(24):
                    nc.sync.dma_start(out=w2[:, k], in_=fc2w[k])
                b2c = f2w.tile([128, 768], f32)
                nc.sync.dma_start(out=b2c, in_=fc2b_bc)
                for (t0, ts) in CHUNKS:
                    h3 = f2a.tile([128, 24, 128], bf16, tag="f2h")
                    for k in range(24):
                        nc.sync.dma_start(out=h3[:, k, :ts], in_=h3T_d[k, :, t0:t0 + ts])
                    y2 = f2a.tile([128, 768], f32, tag="f2y")
                    nc.sync.dma_start(out=y2[:ts], in_=y2_d[t0:t0 + ts, :])
                    ot = f2a.tile([128, 768], f32, tag="f2o")
                    ps0 = f2p.tile([128, 384], f32, tag="f2ps")
                    ps1 = f2p.tile([128, 384], f32, tag="f2ps")
                    for k in range(24):
                        nc.tensor.matmul(ps0[:ts], h3[:, k, :ts], w2[:, k, 0:384],
                                         start=(k == 0), stop=(k == 23))
                        nc.tensor.matmul(ps1[:ts], h3[:, k, :ts], w2[:, k, 384:768],
                                         start=(k == 0), stop=(k == 23))
                    nc.vector.tensor_add(ot[:ts, 0:384], ps0[:ts], y2[:ts, 0:384])
                    nc.vector.tensor_add(ot[:ts, 384:768], ps1[:ts], y2[:ts, 384:768])
                    nc.gpsimd.tensor_add(ot[:ts], ot[:ts], b2c[:ts])
                    nc.sync.dma_start(out=out_d[t0:t0 + ts, :], in_=ot[:ts])

    nc.compile()
    return nc


def kernel(x, H, W, geo_bias, extra_token, conv_w, conv_b, pe_g, pe_b,
           n1_g, n1_b, qkv_w, rpb_table, proj_w, proj_b, n2_g, n2_b,
           fc1_w, fc1_b, fc2_w, fc2_b):
    x = np.asarray(x, np.float32)
    f = lambda a: np.asarray(a, np.float32)
    geo_bias, extra_token = f(geo_bias), f(extra_token)
    conv_w, conv_b, pe_g, pe_b = f(conv_w), f(conv_b), f(pe_g), f(pe_b)
    n1_g, n1_b, qkv_w, rpb_table = f(n1_g), f(n1_b), f(qkv_w), f(rpb_table)
    proj_w, proj_b, n2_g, n2_b = f(proj_w), f(proj_b), f(n2_g), f(n2_b)
    fc1_w, fc1_b, fc2_w, fc2_b = f(fc1_w), f(fc1_b), f(fc2_w), f(fc2_b)

    if "nc" not in _CACHE:
        _CACHE["nc"] = build()
    nc = _CACHE["nc"]

    # host-side weight prep (layout only; LN scale folds are exact for g=1,b=0)
    cw = conv_w.transpose(2, 3, 1, 0).reshape(3, 3, 3, 128, 768).reshape(27, 128, 768)
    qkv_wf = qkv_w * n1_g[None, :]
    qkv_wf[:D] *= HD ** -0.5
    qkv_b = qkv_w @ n1_b
    qkv_b[:D] *= HD ** -0.5
    fc1_wf = fc1_w * n2_g[None, :]
    fc1_bf = fc1_b + fc1_w @ n2_b
    bias_full = _rel_bias(rpb_table)
    bT = np.zeros((12, 128, 3, 257), np.float32)
    for mi, (mo, ms) in enumerate(MCH):
        bT[:, :ms, mi, :] = bias_full[:, :, mo:mo + ms].transpose(0, 2, 1)
    bT = np.exp(bT)

    common = {
        "convw": cw.astype(BF),
        "convb_bc": np.tile(conv_b[None, :], (128, 1)).astype(np.float32),
        "peg_bc": np.tile(pe_g[None, :], (128, 1)).astype(np.float32),
        "geo2": (geo_bias[0, 1:, :] + pe_b[None, :]).reshape(2, 128, 768).astype(np.float32),
        "y0row": (extra_token[0] + geo_bias[0, :1, :]).astype(np.float32),
        "qkvw": qkv_wf.T.reshape(6, 128, 2304).astype(BF),
        "qkvb_t": np.ascontiguousarray(qkv_b.reshape(18, 128).T).astype(np.float32),
        "projw": proj_w.T.reshape(6, 128, 768).astype(BF),
        "projb_bc": np.tile(proj_b[None, :], (128, 1)).astype(np.float32),
        "fc1w": fc1_wf.T.reshape(6, 128, MLP).astype(BF),
        "fc1b_t": np.ascontiguousarray(fc1_bf.reshape(24, 128).T).astype(np.float32),
        "fc2w": fc2_w.T.reshape(24, 128, 768).astype(BF),
        "fc2b_bc": np.tile(fc2_b[None, :], (128, 1)).astype(np.float32),
        "biasT": bT.astype(BF),
        "ones_d": np.ones((1, 257), BF),
        "e2_d": np.kron(np.eye(2), np.ones((1, 64))).astype(BF),
    }
    in_maps = []
    for c in range(8):
        xs = x[c * BL:(c + 1) * BL].reshape(BL, 3, 128, 32, 32).astype(BF)
        in_maps.append({"x_in": xs, **common})

    _CACHE["maps"] = in_maps
    res = bass_utils.run_bass_kernel_spmd(nc, in_maps, core_ids=list(range(8)))
    out = np.concatenate([r["out_d"].reshape(BL, N, D) for r in res.results], axis=0)
    return out.astype(np.float32)



# revision 28
# speedup vs baseline: 1.1146x; 1.1146x over previous
"""MHSA block (patch-embed conv + relative-pos attention + MLP) on 8 NeuronCores.

Sharding: pure data-parallel over batch (64 images -> 8 per core), weights
replicated. Host does only layout prep (transposes/casts/rel-pos gather);
all model compute runs on-device via Bass/Tile.
"""
import numpy as np
import ml_dtypes
import concourse.bass as bass
import concourse.bacc as bacc
import concourse.tile as tile
from concourse import mybir
from concourse import bass_utils
from concourse.masks import make_identity

BF = ml_dtypes.bfloat16
B, CIN, D, HEADS, HD = 64, 384, 768, 12, 64
GS, ET, N = 16, 1, 257
BL = B // 8              # images per core
NT = BL * N              # 2056 packed tokens per core
MLP = 4 * D
CHUNKS = [(i * 128, min(128, NT - i * 128)) for i in range(17)]
COLT = [(c, min(512, NT - c)) for c in range(0, NT, 512)]
MCH = [(0, 128), (128, 128), (256, 1)]

_CACHE = {}


def _rel_bias(rpb_table):
    coords = np.stack(np.meshgrid(np.arange(GS), np.arange(GS), indexing='ij'))
    cf = coords.reshape(2, -1)
    rel = (cf[:, :, None] - cf[:, None, :]).transpose(1, 2, 0)
    rel[:, :, 0] += GS - 1
    rel[:, :, 1] += GS - 1
    rel[:, :, 0] *= 2 * GS - 1
    idx = rel.sum(-1)
    out = np.zeros((N, N), dtype=np.int32)
    out[ET:, ET:] = idx
    bias = rpb_table[out]                    # [N, N, HEADS]
    return bias.transpose(2, 0, 1).astype(np.float32)   # [HEADS, N, N]


def _ln_pair(tc, nc, pools, xt, ts):
    """mean/rstd of xt[:ts, :768] -> (mean, rstd) [ts,1] f32 tiles."""
    st = pools.tile([128, 3, nc.vector.BN_STATS_DIM], mybir.dt.float32, tag="lnst")
    xg = xt.rearrange("p (n f) -> p n f", f=256)
    for i in range(3):
        nc.vector.bn_stats(out=st[:ts, i], in_=xg[:ts, i])
    mv = pools.tile([128, nc.vector.BN_AGGR_DIM], mybir.dt.float32, tag="lnmv")
    nc.vector.bn_aggr(out=mv[:ts], in_=st[:ts])
    eps = pools.tile([128, 1], mybir.dt.float32, tag="lneps")
    nc.vector.memset(eps, 1e-5)
    rs = pools.tile([128, 1], mybir.dt.float32, tag="lnrs")
    nc.scalar.activation(out=rs[:ts], in_=mv[:ts, 1:2],
                         func=mybir.ActivationFunctionType.Sqrt, bias=eps[:ts])
    nc.vector.reciprocal(out=rs[:ts], in_=rs[:ts])
    return mv, rs


def build():
    nc = bacc.Bacc("TRN2", target_bir_lowering=False, debug=False)
    f32, bf16 = mybir.dt.float32, mybir.dt.bfloat16
    di = lambda n, s, d: nc.dram_tensor(n, s, d, kind="ExternalInput").ap()
    x_in = di("x_in", [BL, 3, 128, 32, 32], bf16)
    convw = di("convw", [27, 128, 768], bf16)
    convb_bc = di("convb_bc", [128, 768], f32)
    peg_bc = di("peg_bc", [128, 768], f32)
    geo2 = di("geo2", [2, 128, 768], f32)
    y0row = di("y0row", [1, 768], f32)
    qkvw = di("qkvw", [6, 128, 2304], bf16)
    qkvb_t = di("qkvb_t", [128, 18], f32)
    projw = di("projw", [6, 128, 768], bf16)
    projb_bc = di("projb_bc", [128, 768], f32)
    fc1w = di("fc1w", [6, 128, MLP], bf16)
    fc1b_t = di("fc1b_t", [128, 24], f32)
    fc2w = di("fc2w", [24, 128, 768], bf16)
    fc2b_bc = di("fc2b_bc", [128, 768], f32)
    biasT = di("biasT", [12, 128, 3, 257], bf16)
    ones_d = di("ones_d", [1, 257], bf16)
    e2_d = di("e2_d", [2, 128], bf16)
    out_d = nc.dram_tensor("out_d", [NT, 768], f32, kind="ExternalOutput").ap()

    with tile.TileContext(nc) as tc:
        with tc.tile_pool(name="dram", bufs=1, space="DRAM") as dpool:
            y_d = dpool.tile([NT, 768], f32)
            hT_d = dpool.tile([6, 128, NT], bf16)
            qkT_d = dpool.tile([18, 128, NT], bf16)
            oT_d = dpool.tile([6, 128, NT], bf16)
            y2_d = dpool.tile([NT, 768], f32)
            den_d = dpool.tile([12, NT], bf16)
            h2T_d = dpool.tile([6, 128, NT], bf16)
            h3T_d = dpool.tile([24, 128, NT], bf16)

            # ---------------- Phase 1: conv + peLN + geo -> y_d ----------------
            with tc.tile_pool(name="cw", bufs=1) as cw, \
                 tc.tile_pool(name="cx", bufs=2) as cx, \
                 tc.tile_pool(name="cps", bufs=4, space="PSUM") as cps, \
                 tc.tile_pool(name="cy", bufs=3) as cy:
                wsb = cw.tile([128, 27, 768], bf16)
                for i in range(27):
                    nc.sync.dma_start(out=wsb[:, i], in_=convw[i])
                cbc = cw.tile([128, 768], f32)
                nc.sync.dma_start(out=cbc, in_=convb_bc)
                pgc = cw.tile([128, 768], f32)
                nc.sync.dma_start(out=pgc, in_=peg_bc)
                gsb = cw.tile([128, 2, 768], f32)
                for t in range(2):
                    nc.sync.dma_start(out=gsb[:, t], in_=geo2[t])
                y0sb = cw.tile([1, 768], f32)
                nc.sync.dma_start(out=y0sb, in_=y0row)
                for b in range(BL):
                    nc.sync.dma_start(out=y_d[b * N:b * N + 1, :], in_=y0sb)
                for b in range(BL):
                    xp = cx.tile([128, 3, 1089], bf16, tag="xpad")
                    nc.vector.memset(xp, 0.0)
                    for c in range(3):
                        dst = bass.AP(tensor=xp.tensor, offset=xp.offset + c * 1089 + 34,
                                      ap=[xp.ap[0], [33, 32], [1, 32]])
                        nc.sync.dma_start(out=dst, in_=x_in[b, c])
                    for t in range(2):
                        col = cx.tile([128, 27, 128], bf16, tag="col")
                        for kh in range(3):
                            for kw in range(3):
                                for c in range(3):
                                    idx = (kh * 3 + kw) * 3 + c
                                    src = bass.AP(
                                        tensor=xp.tensor,
                                        offset=xp.offset + c * 1089 + (16 * t + kh) * 33 + kw,
                                        ap=[xp.ap[0], [66, 8], [2, 16]])
                                    if c == 0:
                                        nc.vector.tensor_copy(col[:, idx], src)
                                    elif c == 1:
                                        nc.gpsimd.tensor_copy(col[:, idx], src)
                                    else:
                                        nc.scalar.copy(col[:, idx], src)
                        yt = cy.tile([128, 768], f32, tag="yt")
                        for nh in range(2):
                            ps = cps.tile([128, 384], f32, tag="cpsum")
                            for i in range(27):
                                nc.tensor.matmul(ps, col[:, i], wsb[:, i, nh * 384:(nh + 1) * 384],
                                                 start=(i == 0), stop=(i == 26))
                            nc.vector.tensor_add(yt[:, nh * 384:(nh + 1) * 384], ps, cbc[:, nh * 384:(nh + 1) * 384])
                        mv, rs = _ln_pair(tc, nc, cy, yt, 128)
                        nc.vector.tensor_scalar(out=yt, in0=yt, scalar1=mv[:, 0:1], scalar2=rs,
                                                op0=mybir.AluOpType.subtract, op1=mybir.AluOpType.mult)
                        nc.gpsimd.tensor_mul(yt, yt, pgc)
                        nc.vector.tensor_add(yt, yt, gsb[:, t])
                        r0 = b * N + 1 + t * 128
                        nc.sync.dma_start(out=y_d[r0:r0 + 128, :], in_=yt)

            # ---------------- Phase 2: LN1 + transpose -> hT_d ----------------
            with tc.tile_pool(name="l1", bufs=3) as l1, \
                 tc.tile_pool(name="l1c", bufs=1) as l1c, \
                 tc.tile_pool(name="l1p", bufs=4, space="PSUM") as l1p:
                idb = l1c.tile([128, 128], bf16)
                make_identity(nc, idb)
                for (t0, ts) in CHUNKS:
                    yt = l1.tile([128, 768], f32, tag="l1y")
                    nc.sync.dma_start(out=yt[:ts], in_=y_d[t0:t0 + ts, :])
                    mv, rs = _ln_pair(tc, nc, l1, yt, ts)
                    hb = l1.tile([128, 768], bf16, tag="l1h")
                    nc.vector.tensor_scalar(out=hb[:ts], in0=yt[:ts], scalar1=mv[:ts, 0:1], scalar2=rs[:ts],
                                            op0=mybir.AluOpType.subtract, op1=mybir.AluOpType.mult)
                    for k in range(6):
                        tp = l1p.tile([128, 128], bf16, tag="l1t")
                        nc.tensor.transpose(tp[:, :ts], hb[:ts, k * 128:(k + 1) * 128], idb[:ts, :ts])
                        st = l1.tile([128, 128], bf16, tag="l1s")
                        nc.vector.tensor_copy(st[:, :ts], tp[:, :ts])
                        nc.sync.dma_start(out=hT_d[k, :, t0:t0 + ts], in_=st[:, :ts])

            # ---------------- Phase 3: QKV -> qkT_d ----------------
            with tc.tile_pool(name="qw", bufs=1) as qw, \
                 tc.tile_pool(name="qa", bufs=3) as qa, \
                 tc.tile_pool(name="qp", bufs=8, space="PSUM") as qp:
                wq = qw.tile([128, 6, 2304], bf16)
                for k in range(6):
                    nc.sync.dma_start(out=wq[:, k], in_=qkvw[k])
                qb = qw.tile([128, 18], f32)
                nc.sync.dma_start(out=qb, in_=qkvb_t)
                for (c0, cs) in COLT:
                    ht = qa.tile([128, 6, 512], bf16, tag="qh")
                    for k in range(6):
                        nc.sync.dma_start(out=ht[:, k, :cs], in_=hT_d[k, :, c0:c0 + cs])
                    for dch in range(18):
                        ps = qp.tile([128, 512], f32, tag="qps")
                        for k in range(6):
                            nc.tensor.matmul(ps[:, :cs], wq[:, k, dch * 128:(dch + 1) * 128],
                                             ht[:, k, :cs], start=(k == 0), stop=(k == 5))
                        ev = qa.tile([128, 512], bf16, tag="qev")
                        nc.vector.tensor_scalar_add(out=ev[:, :cs], in0=ps[:, :cs], scalar1=qb[:, dch:dch + 1])
                        nc.sync.dma_start(out=qkT_d[dch, :, c0:c0 + cs], in_=ev[:, :cs])

            # ---------------- Phase 4: attention (S^T orientation) -> oT_d ----------------
            # S^T [keys, queries] = K Q^T + bias^T (bias injected via identity-
            # stationary matmul into the same PSUM). The softmax denominator
            # comes free from a ones row appended to V before its transpose.
            # o^T is written UNNORMALIZED; denominators go to den_d and the
            # per-query 1/denom scaling happens batched in phase 5.
            with tc.tile_pool(name="ac", bufs=1) as ac, \
                 tc.tile_pool(name="ab", bufs=2) as ab, \
                 tc.tile_pool(name="aw", bufs=3) as aw, \
                 tc.tile_pool(name="aps", bufs=3, space="PSUM") as aps, \
                 tc.tile_pool(name="apt", bufs=2, space="PSUM") as apt, \
                 tc.tile_pool(name="apo", bufs=1, space="PSUM") as apo:
                idb = ac.tile([128, 128], bf16)
                make_identity(nc, idb)
                for h in range(12):
                    bsb = ab.tile([128, 3, 257], bf16, tag="bias")
                    nc.sync.dma_start(out=bsb, in_=biasT[h])
                    for b in range(BL):
                        po = (h % 2) * 64
                        qt = aw.tile([64, 257], bf16, tag="qt")
                        kt = aw.tile([64, 257], bf16, tag="kt")
                        vt = aw.tile([65, 257], bf16, tag="vt")
                        nc.sync.dma_start(out=qt, in_=qkT_d[h // 2, po:po + 64, b * N:b * N + N])
                        nc.sync.dma_start(out=kt, in_=qkT_d[6 + h // 2, po:po + 64, b * N:b * N + N])
                        nc.sync.dma_start(out=vt[:64], in_=qkT_d[12 + h // 2, po:po + 64, b * N:b * N + N])
                        nc.sync.dma_start(out=vt[64:65], in_=ones_d)
                        vsb = aw.tile([128, 3, 65], bf16, tag="vsb")
                        for mi, (mo, ms) in enumerate(MCH):
                            tp = apt.tile([128, 65], bf16, tag="vtp")
                            nc.tensor.transpose(tp[:ms], vt[:, mo:mo + ms], idb[:65, :65])
                            nc.vector.tensor_copy(vsb[:ms, mi], tp[:ms])
                        pr = aw.tile([128, 3, 257], bf16, tag="pr")
                        for mi, (mo, ms) in enumerate(MCH):
                            sp = aps.tile([128, 257], f32, tag="sps")
                            nc.tensor.matmul(sp[:ms], kt[:, mo:mo + ms], qt,
                                             start=True, stop=True)
                            nc.scalar.activation(pr[:ms, mi], sp[:ms],
                                                 mybir.ActivationFunctionType.Exp)
                            # exp(s + b) = exp(s) * exp(b); bsb holds exp(bias^T)
                            nc.gpsimd.tensor_mul(pr[:ms, mi], pr[:ms, mi], bsb[:ms, mi])
                        op = apo.tile([65, 257], f32, tag="ops", bufs=2)
                        for mi, (mo, ms) in enumerate(MCH):
                            nc.tensor.matmul(op, vsb[:ms, mi], pr[:ms, mi],
                                             start=(mi == 0), stop=(mi == 2))
                        oe = aw.tile([65, 257], bf16, tag="oe")
                        nc.vector.tensor_copy(oe, op)
                        nc.sync.dma_start(out=oT_d[h // 2, po:po + 64, b * N:b * N + N], in_=oe[:64])
                        nc.sync.dma_start(out=den_d[h:h + 1, b * N:b * N + N], in_=oe[64:65])

            # ---------------- Phase 5: proj + residual + LN2 + T -> y2_d, h2T_d ----------------
            with tc.tile_pool(name="pw", bufs=1) as pw, \
                 tc.tile_pool(name="pa", bufs=3) as pa, \
                 tc.tile_pool(name="pp", bufs=3, space="PSUM") as pp:
                wp = pw.tile([128, 6, 768], bf16)
                for k in range(6):
                    nc.sync.dma_start(out=wp[:, k], in_=projw[k])
                pbc = pw.tile([128, 768], f32)
                nc.sync.dma_start(out=pbc, in_=projb_bc)
                idb2 = pw.tile([128, 128], bf16)
                make_identity(nc, idb2)
                # batched softmax denominators: one reciprocal over [12, NT]
                dsb = pw.tile([12, NT], bf16)
                nc.sync.dma_start(out=dsb, in_=den_d)
                rcb = pw.tile([12, NT], f32)
                nc.vector.reciprocal(rcb, dsb)
                rch = pw.tile([12, NT], bf16)
                nc.vector.tensor_copy(rch, rcb)
                # head-pair rows (2k, 2k+1) at partitions (0, 1) for k=0..5
                den2 = pw.tile([2, 6, NT], bf16)
                for k in range(6):
                    nc.sync.dma_start(out=den2[0:1, k], in_=rch[2 * k:2 * k + 1])
                    nc.sync.dma_start(out=den2[1:2, k], in_=rch[2 * k + 1:2 * k + 2])
                e2 = pw.tile([2, 128], bf16)
                nc.sync.dma_start(out=e2, in_=e2_d)
                for (t0, ts) in CHUNKS:
                    ot = pa.tile([128, 6, 128], bf16, tag="pot")
                    for k in range(6):
                        nc.sync.dma_start(out=ot[:, k, :ts], in_=oT_d[k, :, t0:t0 + ts])
                    for k in range(6):
                        sden = pp.tile([128, 128], f32, tag="sden", bufs=2)
                        nc.tensor.matmul(sden[:, :ts], e2, den2[:, k, t0:t0 + ts],
                                         start=True, stop=True)
                        nc.vector.tensor_mul(ot[:, k, :ts], ot[:, k, :ts], sden[:, :ts])
                    yt = pa.tile([128, 768], f32, tag="py")
                    nc.sync.dma_start(out=yt[:ts], in_=y_d[t0:t0 + ts, :])
                    y2 = pa.tile([128, 768], f32, tag="py2")
                    for nh in range(2):
                        ps = pp.tile([128, 384], f32, tag="pps")
                        for k in range(6):
                            nc.tensor.matmul(ps[:ts], ot[:, k, :ts], wp[:, k, nh * 384:(nh + 1) * 384],
                                             start=(k == 0), stop=(k == 5))
                        nc.vector.tensor_add(y2[:ts, nh * 384:(nh + 1) * 384], ps[:ts],
                                             yt[:ts, nh * 384:(nh + 1) * 384])
                    nc.vector.tensor_add(y2[:ts], y2[:ts], pbc[:ts])
                    nc.sync.dma_start(out=y2_d[t0:t0 + ts, :], in_=y2[:ts])
                    mv, rs = _ln_pair(tc, nc, pa, y2, ts)
                    hb = pa.tile([128, 768], bf16, tag="ph2")
                    nc.vector.tensor_scalar(out=hb[:ts], in0=y2[:ts], scalar1=mv[:ts, 0:1], scalar2=rs[:ts],
                                            op0=mybir.AluOpType.subtract, op1=mybir.AluOpType.mult)
                    for k in range(6):
                        tp = pp.tile([128, 128], bf16, tag="ptr")
                        nc.tensor.transpose(tp[:, :ts], hb[:ts, k * 128:(k + 1) * 128], idb2[:ts, :ts])
                        st = pa.tile([128, 128], bf16, tag="pst")
                        nc.vector.tensor_copy(st[:, :ts], tp[:, :ts])
                        nc.sync.dma_start(out=h2T_d[k, :, t0:t0 + ts], in_=st[:, :ts])

            # ---------------- Phase 6: FC1 + gelu -> h3T_d ----------------
            with tc.tile_pool(name="f1w", bufs=1) as f1w, \
                 tc.tile_pool(name="f1a", bufs=3) as f1a, \
                 tc.tile_pool(name="f1p", bufs=8, space="PSUM") as f1p:
                w1 = f1w.tile([128, 6, MLP], bf16)
                for k in range(6):
                    nc.sync.dma_start(out=w1[:, k], in_=fc1w[k])
                b1 = f1w.tile([128, 24], f32)
                nc.sync.dma_start(out=b1, in_=fc1b_t)
                for (c0, cs) in COLT:
                    ht = f1a.tile([128, 6, 512], bf16, tag="f1h")
                    for k in range(6):
                        nc.sync.dma_start(out=ht[:, k, :cs], in_=h2T_d[k, :, c0:c0 + cs])
                    for dch in range(24):
                        ps = f1p.tile([128, 512], f32, tag="f1ps")
                        for k in range(6):
                            nc.tensor.matmul(ps[:, :cs], w1[:, k, dch * 128:(dch + 1) * 128],
                                             ht[:, k, :cs], start=(k == 0), stop=(k == 5))
                        ev = f1a.tile([128, 512], bf16, tag="f1ev")
                        nc.scalar.activation(ev[:, :cs], ps[:, :cs], mybir.ActivationFunctionType.Gelu,
                                             bias=b1[:, dch:dch + 1])
                        nc.sync.dma_start(out=h3T_d[dch, :, c0:c0 + cs], in_=ev[:, :cs])

            # ---------------- Phase 7: FC2 + residual -> out ----------------
            with tc.tile_pool(name="f2w", bufs=1) as f2w, \
                 tc.tile_pool(name="f2a", bufs=3) as f2a, \
                 tc.tile_pool(name="f2p", bufs=8, space="PSUM") as f2p:
                w2 = f2w.tile([128, 24, 768], bf16)
                for k in range(24):
                    nc.sync.dma_start(out=w2[:, k], in_=fc2w[k])
                b2c = f2w.tile([128, 768], f32)
                nc.sync.dma_start(out=b2c, in_=fc2b_bc)
                for (t0, ts) in CHUNKS:
                    h3 = f2a.tile([128, 24, 128], bf16, tag="f2h")
                    for k in range(24):
                        nc.sync.dma_start(out=h3[:, k, :ts], in_=h3T_d[k, :, t0:t0 + ts])
                    y2 = f2a.tile([128, 768], f32, tag="f2y")
                    nc.sync.dma_start(out=y2[:ts], in_=y2_d[t0:t0 + ts, :])
                    ot = f2a.tile([128, 768], f32, tag="f2o")
                    for nh in range(2):
                        ps = f2p.tile([128, 384], f32, tag="f2ps")
                        for k in range(24):
                            nc.tensor.matmul(ps[:ts], h3[:, k, :ts], w2[:, k, nh * 384:(nh + 1) * 384],
                                             start=(k == 0), stop=(k == 23))
                        nc.vector.tensor_add(ot[:ts, nh * 384:(nh + 1) * 384], ps[:ts],
                                             y2[:ts, nh * 384:(nh + 1) * 384])
                    nc.gpsimd.tensor_add(ot[:ts], ot[:ts], b2c[:ts])
                    nc.sync.dma_start(out=out_d[t0:t0 + ts, :], in_=ot[:ts])

    nc.compile()
    return nc


def kernel(x, H, W, geo_bias, extra_token, conv_w, conv_b, pe_g, pe_b,
           n1_g, n1_b, qkv_w, rpb_table, proj_w, proj_b, n2_g, n2_b,
           fc1_w, fc1_b, fc2_w, fc2_b):
    x = np.asarray(x, np.float32)
    f = lambda a: np.asarray(a, np.float32)
    geo_bias, extra_token = f(geo_bias), f(extra_token)
    conv_w, conv_b, pe_g, pe_b = f(conv_w), f(conv_b), f(pe_g), f(pe_b)
    n1_g, n1_b, qkv_w, rpb_table = f(n1_g), f(n1_b), f(qkv_w), f(rpb_table)
    proj_w, proj_b, n2_g, n2_b = f(proj_w), f(proj_b), f(n2_g), f(n2_b)
    fc1_w, fc1_b, fc2_w, fc2_b = f(fc1_w), f(fc1_b), f(fc2_w), f(fc2_b)

    if "nc" not in _CACHE:
        _CACHE["nc"] = build()
    nc = _CACHE["nc"]

    # host-side weight prep (layout only; LN scale folds are exact for g=1,b=0)
    cw = conv_w.transpose(2, 3, 1, 0).reshape(3, 3, 3, 128, 768).reshape(27, 128, 768)
    qkv_wf = qkv_w * n1_g[None, :]
    qkv_wf[:D] *= HD ** -0.5
    qkv_b = qkv_w @ n1_b
    qkv_b[:D] *= HD ** -0.5
    fc1_wf = fc1_w * n2_g[None, :]
    fc1_bf = fc1_b + fc1_w @ n2_b
    bias_full = _rel_bias(rpb_table)
    bT = np.zeros((12, 128, 3, 257), np.float32)
    for mi, (mo, ms) in enumerate(MCH):
        bT[:, :ms, mi, :] = bias_full[:, :, mo:mo + ms].transpose(0, 2, 1)
    bT = np.exp(bT)

    common = {
        "convw": cw.astype(BF),
        "convb_bc": np.tile(conv_b[None, :], (128, 1)).astype(np.float32),
        "peg_bc": np.tile(pe_g[None, :], (128, 1)).astype(np.float32),
        "geo2": (geo_bias[0, 1:, :] + pe_b[None, :]).reshape(2, 128, 768).astype(np.float32),
        "y0row": (extra_token[0] + geo_bias[0, :1, :]).astype(np.float32),
        "qkvw": qkv_wf.T.reshape(6, 128, 2304).astype(BF),
        "qkvb_t": np.ascontiguousarray(qkv_b.reshape(18, 128).T).astype(np.float32),
        "projw": proj_w.T.reshape(6, 128, 768).astype(BF),
        "projb_bc": np.tile(proj_b[None, :], (128, 1)).astype(np.float32),
        "fc1w": fc1_wf.T.reshape(6, 128, MLP).astype(BF),
        "fc1b_t": np.ascontiguousarray(fc1_bf.reshape(24, 128).T).astype(np.float32),
        "fc2w": fc2_w.T.reshape(24, 128, 768).astype(BF),
        "fc2b_bc": np.tile(fc2_b[None, :], (128, 1)).astype(np.float32),
        "biasT": bT.astype(BF),
        "ones_d": np.ones((1, 257), BF),
        "e2_d": np.kron(np.eye(2), np.ones((1, 64))).astype(BF),
    }
    in_maps = []
    for c in range(8):
        xs = x[c * BL:(c + 1) * BL].reshape(BL, 3, 128, 32, 32).astype(BF)
        in_maps.append({"x_in": xs, **common})

    _CACHE["maps"] = in_maps
    res = bass_utils.run_bass_kernel_spmd(nc, in_maps, core_ids=list(range(8)))
    out = np.concatenate([r["out_d"].reshape(BL, N, D) for r in res.results], axis=0)
    return out.astype(np.float32)

